# revision 1
# baseline (speedup 1.0000x reference)
"""Trainium2 Bass kernel for nn_AutoregressiveGaussian.

Model: noise-MLP -> LSTM-style autoregressive sampler, S=512 steps,
B=4096 batch, F=128 features, D=256 hidden.

Strategy: pure data parallel over 8 NeuronCores (512 batch rows each).
On-device layout keeps features on SBUF partitions and batch on the free
dim, so every matmul is out[featT] = W.T-chunks @ actT with zero
transposes.  Host pre-transposes eps/noise and post-transposes the
output (not part of HW time).  Within a core the batch is split into two
interleaved streams of 256 so the serial LSTM dependency chain of one
stream overlaps the other stream's engine work.

Numerics: the x-path matmuls run in float32r (full PE rate, ~1.5e-4
rel); the recurrent c/h path (gate activations, cell state, h, and the
w_hh/out_w matmuls) runs in bf16, which halves DVE tensor-tensor cost
(2x_1p mode) -- the LSTM's contractive gates keep the accumulated error
at ~2.6e-4 end to end.  Activations all come from one ACT table set
(sigmoid/tanh/erf).  exp(ls) is emulated as (1+t)/(1-t), t=tanh(ls/2),
with the fast approx reciprocal on DVE, avoiding a per-step ~2.7us ACT
table switch to the exp set.  gelu(x)=0.5*x*(1+erf(x/sqrt2)) exactly
matches the reference's erf gelu; the 0.5 is folded into consumer
weights on the host.  Steps are emitted as op-interleaved stages across
the two streams; PSUM is partitioned into release-time-matched pools
(3x2-bank gate slots + 2x1-bank tail slots) so next-step w_hh matmuls
prefetch while the current tail drains.
"""
import sys
sys.path.insert(0, "/opt/trn_rl_repo")

import numpy as np

B, S_FULL, F = 4096, 512, 128
D = 2 * F
NCORES = 8
BL = B // NCORES          # 512 rows per core
NS = 2                    # streams per core
NB = BL // NS             # 256 batch rows per stream (free dim)

SQ2I = float(1.0 / np.sqrt(2.0))


def _build(S, bias_flags):
    import concourse.bacc as bacc
    import concourse.mybir as mybir
    import concourse.tile as tile

    F32 = mybir.dt.float32
    F32R = mybir.dt.float32r
    BF16 = mybir.dt.bfloat16
    AF = mybir.ActivationFunctionType
    OP = mybir.AluOpType

    gb_nz, outb_nz, zbmu_nz, zbls_nz, mlpb_nz, hidb_nz = bias_flags

    nc = bacc.Bacc("TRN2", target_bir_lowering=False, debug=False,
                   num_devices=NCORES)

    eps_d = nc.dram_tensor("epsT", [S, F, BL], F32, kind="ExternalInput").ap()
    noise_d = nc.dram_tensor("noiseT", [F, BL], F32R, kind="ExternalInput").ap()
    wih_d = nc.dram_tensor("wih", [F, 4 * D], F32R, kind="ExternalInput").ap()
    whh_d = nc.dram_tensor("whh", [D, 4 * D], BF16, kind="ExternalInput").ap()
    outw_d = nc.dram_tensor("outw", [D, F], BF16, kind="ExternalInput").ap()
    zw_d = nc.dram_tensor("zw", [F, 2 * F], F32R, kind="ExternalInput").ap()
    mlp_d = nc.dram_tensor("mlp", [F, 3 * F + D], F32R, kind="ExternalInput").ap()
    bias_d = nc.dram_tensor("biaspack", [F, 16], F32, kind="ExternalInput").ap()
    out_d = nc.dram_tensor("outT", [S, F, BL], F32R, kind="ExternalOutput").ap()

    with tile.TileContext(nc) as tc:
        with tc.tile_pool(name="const", bufs=1) as cp, \
             tc.tile_pool(name="eps", bufs=8) as ep, \
             tc.tile_pool(name="state", bufs=3) as sp, \
             tc.tile_pool(name="gates", bufs=2) as gp, \
             tc.tile_pool(name="tail", bufs=3) as tp, \
             tc.tile_pool(name="ps", bufs=3, space="PSUM") as pp, \
             tc.tile_pool(name="pst", bufs=2, space="PSUM") as pst:

            # ---- constants ----
            wih_t = cp.tile([F, 4 * D], F32R, tag="wih")
            nc.gpsimd.dma_start(wih_t[:], wih_d)
            whh_t = cp.tile([128, 2 * 4 * D], BF16, tag="whh2")
            nc.gpsimd.dma_start(whh_t[:, 0:4 * D], whh_d[0:128, :])
            nc.gpsimd.dma_start(whh_t[:, 4 * D:8 * D], whh_d[128:256, :])
            outw_t = cp.tile([128, 2 * F], BF16, tag="outw")
            nc.gpsimd.dma_start(outw_t[:, 0:F], outw_d[0:128, :])
            nc.gpsimd.dma_start(outw_t[:, F:2 * F], outw_d[128:256, :])
            zw_t = cp.tile([F, 2 * F], F32R, tag="zw")
            nc.gpsimd.dma_start(zw_t[:], zw_d)
            mlp_t = cp.tile([F, 3 * F + D], F32R, tag="mlp")
            nc.gpsimd.dma_start(mlp_t[:], mlp_d)
            bias_t = cp.tile([F, 16], F32, tag="bias")
            nc.gpsimd.dma_start(bias_t[:], bias_d)
            noise_t = cp.tile([F, BL], F32R, tag="noise")
            nc.gpsimd.dma_start(noise_t[:], noise_d)

            def bcol(j):
                return bias_t[:, j:j + 1]
            # bias pack: 0-7 gates chunks, 8 out_b, 9 zb_mu, 10 0.5*zb_ls,
            # 11-13 mlp_b1/2/3, 14-15 hid_b chunks

            xT = [None] * NS
            hT = [None] * NS
            cT = [None] * NS

            # ---- prologue (per stream): noise MLP + initial h,c ----
            for si in range(NS):
                nsl = noise_t[:, si * NB:(si + 1) * NB]

                def gelu_layer(x_rhs, w_lhsT, b_idx, b_nz, tag):
                    ps = pp.tile([128, 4 * NB], F32, tag="ps", name=f"psml_{tag}")
                    nc.tensor.matmul(ps[:, 0:NB], w_lhsT, x_rhs,
                                     start=True, stop=True)
                    if b_nz:
                        ob = tp.tile([128, NB], F32, tag=f"ob_{tag}",
                                     name=f"ob_{tag}")
                        nc.vector.tensor_scalar_add(ob[:], ps[:, 0:NB], bcol(b_idx))
                        src = ob[:]
                    else:
                        src = ps[:, 0:NB]
                    e = tp.tile([128, NB], F32, tag=f"e_{tag}", name=f"e_{tag}")
                    nc.scalar.activation(e[:], src, AF.Erf, scale=SQ2I)
                    go = sp.tile([128, NB], F32R, tag=f"go_{tag}", name=f"go_{tag}")
                    nc.vector.scalar_tensor_tensor(go[:], e[:], 1.0, src,
                                                   OP.add, OP.mult)
                    return go

                x1 = gelu_layer(nsl, mlp_t[:, 0:F], 11, mlpb_nz, f"m1_{si}")
                x2 = gelu_layer(x1[:], mlp_t[:, F:2 * F], 12, mlpb_nz, f"m2_{si}")
                ps_in = pp.tile([128, 4 * NB], F32, tag="ps", name="ps_in")
                nc.tensor.matmul(ps_in[:, 0:NB], mlp_t[:, 2 * F:3 * F], x2[:],
                                 start=True, stop=True)
                xT[si] = sp.tile([128, NB], F32R, tag=f"xT{si}", name=f"xT{si}")
                if mlpb_nz:
                    nc.vector.tensor_scalar_add(xT[si][:], ps_in[:, 0:NB], bcol(13))
                else:
                    nc.vector.tensor_copy(xT[si][:], ps_in[:, 0:NB])
                for d_ in range(2):
                    nc.tensor.matmul(ps_in[:, NB + d_ * NB:NB + (d_ + 1) * NB],
                                     mlp_t[:, 3 * F + d_ * 128:3 * F + (d_ + 1) * 128],
                                     xT[si][:], start=True, stop=True)
                hT[si] = sp.tile([128, 2 * NB], BF16, tag=f"hT{si}", name=f"hT{si}")
                cT[si] = sp.tile([128, 2 * NB], BF16, tag=f"cT{si}", name=f"cT{si}")
                hsrc = ps_in[:, NB:3 * NB]
                if hidb_nz:
                    for d_ in range(2):
                        nc.vector.tensor_scalar_add(
                            hT[si][:, d_ * NB:(d_ + 1) * NB],
                            ps_in[:, NB + d_ * NB:NB + (d_ + 1) * NB], bcol(14 + d_))
                    nc.scalar.activation(cT[si][:], hT[si][:], AF.Tanh)
                else:
                    nc.vector.tensor_copy(hT[si][:], hsrc)
                    nc.scalar.activation(cT[si][:], hsrc, AF.Tanh)

            # ---- steps ----
            for t in range(S):
                eps_t = ep.tile([128, BL], F32, tag="eps", name="eps_t")
                nc.sync.dma_start(eps_t[:], eps_d[t])

                V = [dict() for _ in range(NS)]

                def gates_mm(si):
                    x_o, h_o = xT[si], hT[si]

                    def gate_mms(ps, gidx_pair):
                        for gi, gidx in enumerate(gidx_pair):
                            for j in range(2):
                                m = gidx * 2 + j
                                col = (gi * 2 + j) * NB
                                for k in range(2):
                                    nc.tensor.matmul(
                                        ps[:, col:col + NB],
                                        whh_t[:, k * 4 * D + m * 128:k * 4 * D + (m + 1) * 128],
                                        h_o[:, k * NB:(k + 1) * NB],
                                        start=(k == 0), stop=False)
                                nc.tensor.matmul(
                                    ps[:, col:col + NB],
                                    wih_t[:, m * 128:(m + 1) * 128],
                                    x_o[:], start=False, stop=True)

                    ps_if = pp.tile([128, 4 * NB], F32, tag="ps", name=f"ps_if{si}")
                    gate_mms(ps_if, (0, 1))
                    ps_og = pp.tile([128, 4 * NB], F32, tag="ps", name=f"ps_og{si}")
                    gate_mms(ps_og, (3, 2))
                    V[si]["ps_if"], V[si]["ps_og"] = ps_if, ps_og

                def acts_if(si):
                    v = V[si]
                    sig_if = gp.tile([128, 4 * NB], BF16, tag=f"sig_if{si}",
                                     name=f"sig_if{si}")
                    if gb_nz:
                        for gi, gidx in ((0, 0), (1, 1)):
                            for j in range(2):
                                m = gidx * 2 + j
                                col = (gi * 2 + j) * NB
                                nc.scalar.activation(
                                    sig_if[:, col:col + NB],
                                    v["ps_if"][:, col:col + NB], AF.Sigmoid,
                                    bias=bcol(m))
                    else:
                        nc.scalar.activation(sig_if[:], v["ps_if"][:], AF.Sigmoid)
                    v["sig_if"] = sig_if

                def acts_og(si):
                    v = V[si]
                    sig_o = gp.tile([128, 2 * NB], BF16, tag=f"sig_o{si}",
                                    name=f"sig_o{si}")
                    gg = gp.tile([128, 2 * NB], BF16, tag=f"gg{si}", name=f"gg{si}")
                    if gb_nz:
                        for gi, gidx, dst, fn in ((0, 3, sig_o, AF.Sigmoid),
                                                  (1, 2, gg, AF.Tanh)):
                            for j in range(2):
                                m = gidx * 2 + j
                                col = (gi * 2 + j) * NB
                                nc.scalar.activation(
                                    dst[:, j * NB:(j + 1) * NB],
                                    v["ps_og"][:, col:col + NB], fn, bias=bcol(m))
                    else:
                        nc.scalar.activation(sig_o[:], v["ps_og"][:, 0:2 * NB],
                                             AF.Sigmoid)
                        nc.scalar.activation(gg[:], v["ps_og"][:, 2 * NB:4 * NB],
                                             AF.Tanh)
                    v["sig_o"], v["gg"] = sig_o, gg

                def dve_u(si):
                    v = V[si]
                    u = gp.tile([128, 2 * NB], BF16, tag=f"u{si}", name=f"u{si}")
                    nc.vector.tensor_mul(u[:], v["sig_if"][:, 2 * NB:4 * NB],
                                         cT[si][:])
                    v["u"] = u

                def dve_t2(si):
                    v = V[si]
                    t2 = gp.tile([128, 2 * NB], BF16, tag=f"t2{si}", name=f"t2{si}")
                    nc.vector.tensor_mul(t2[:], v["sig_if"][:, 0:2 * NB], v["gg"][:])
                    v["t2"] = t2

                def dve_c(si):
                    v = V[si]
                    c_n = sp.tile([128, 2 * NB], BF16, tag=f"cT{si}", name=f"cT{si}")
                    nc.vector.tensor_add(c_n[:], v["u"][:], v["t2"][:])
                    cT[si] = c_n

                def act_th(si):
                    v = V[si]
                    th = gp.tile([128, 2 * NB], BF16, tag=f"th{si}", name=f"th{si}")
                    nc.scalar.activation(th[:], cT[si][:], AF.Tanh)
                    v["th"] = th

                def dve_h(si):
                    v = V[si]
                    h_n = sp.tile([128, 2 * NB], BF16, tag=f"hT{si}", name=f"hT{si}")
                    nc.vector.tensor_mul(h_n[:], v["sig_o"][:], v["th"][:])
                    hT[si] = h_n

                def mm_out(si):
                    v = V[si]
                    tl = pst.tile([128, 2 * NB], F32, tag="pst", name=f"tl{si}")
                    for k in range(2):
                        nc.tensor.matmul(tl[:, 0:NB],
                                         outw_t[:, k * F:(k + 1) * F],
                                         hT[si][:, k * NB:(k + 1) * NB],
                                         start=(k == 0), stop=(k == 1))
                    if outb_nz:
                        obuf = tp.tile([128, NB], F32, tag=f"obuf{si}",
                                       name=f"obuf{si}")
                        nc.vector.tensor_scalar_add(obuf[:], tl[:, 0:NB], bcol(8))
                        v["osrc"] = obuf[:]
                    else:
                        v["osrc"] = tl[:, 0:NB]
                    v["tl"] = tl

                def act_erf(si):
                    v = V[si]
                    e = tp.tile([128, NB], F32, tag=f"e{si}", name=f"e{si}")
                    nc.scalar.activation(e[:], v["osrc"], AF.Erf, scale=SQ2I)
                    v["e"] = e

                def dve_go(si):
                    v = V[si]
                    go = tp.tile([128, NB], F32R, tag=f"go{si}", name=f"go{si}")
                    nc.vector.scalar_tensor_tensor(go[:], v["e"][:], 1.0, v["osrc"],
                                                   OP.add, OP.mult)
                    v["go"] = go

                def mm_z(si):
                    v = V[si]
                    nc.tensor.matmul(v["tl"][:, 0:NB], zw_t[:, 0:F], v["go"][:],
                                     start=True, stop=True)
                    nc.tensor.matmul(v["tl"][:, NB:2 * NB], zw_t[:, F:2 * F],
                                     v["go"][:], start=True, stop=True)

                def act_tls(si):
                    v = V[si]
                    tls = tp.tile([128, NB], F32, tag=f"tls{si}", name=f"tls{si}")
                    if zbls_nz:
                        nc.scalar.activation(tls[:], v["tl"][:, NB:2 * NB], AF.Tanh,
                                             scale=0.5, bias=bcol(10))
                    else:
                        nc.scalar.activation(tls[:], v["tl"][:, NB:2 * NB], AF.Tanh,
                                             scale=0.5)
                    v["tls"] = tls

                def dve_dd(si):
                    v = V[si]
                    dd = tp.tile([128, NB], F32, tag=f"dd{si}", name=f"dd{si}")
                    nc.vector.tensor_scalar(dd[:], v["tls"][:], -1.0, 1.0,
                                            OP.mult, OP.add)
                    v["dd"] = dd

                def dve_r(si):
                    v = V[si]
                    r = tp.tile([128, NB], F32, tag=f"r{si}", name=f"r{si}")
                    nc.vector.reciprocal_approx_fast(r[:], v["dd"][:])
                    v["r"] = r

                def dve_a(si):
                    v = V[si]
                    a = tp.tile([128, NB], F32, tag=f"a{si}", name=f"a{si}")
                    nc.vector.scalar_tensor_tensor(
                        a[:], v["tls"][:], 1.0, eps_t[:, si * NB:(si + 1) * NB],
                        OP.add, OP.mult)
                    v["a"] = a

                def dve_se(si):
                    v = V[si]
                    se = tp.tile([128, NB], F32, tag=f"se{si}", name=f"se{si}")
                    nc.vector.tensor_mul(se[:], v["a"][:], v["r"][:])
                    v["se"] = se

                def dve_sample(si):
                    v = V[si]
                    x_n = sp.tile([128, NB], F32R, tag=f"xT{si}", name=f"xT{si}")
                    if zbmu_nz:
                        xmid = tp.tile([128, NB], F32, tag=f"xmid{si}",
                                       name=f"xmid{si}")
                        nc.vector.tensor_add(xmid[:], v["tl"][:, 0:NB], v["se"][:])
                        nc.vector.tensor_scalar_add(x_n[:], xmid[:], bcol(9))
                    else:
                        nc.vector.tensor_add(x_n[:], v["tl"][:, 0:NB], v["se"][:])
                    nc.sync.dma_start(out_d[t][:, si * NB:(si + 1) * NB], x_n[:])
                    xT[si] = x_n

                for stage in (gates_mm, acts_if, acts_og, dve_u, dve_t2, dve_c,
                              act_th, dve_h, mm_out, act_erf, dve_go, mm_z,
                              act_tls, dve_dd, dve_a, dve_r, dve_se, dve_sample):
                    stage(0)
                    stage(1)

    nc.finalize()
    return nc


def _prep_host(inputs):
    """Shard + transpose inputs on the host; returns per-core in_maps."""
    noise = np.ascontiguousarray(inputs["noise"], dtype=np.float32)
    eps = np.ascontiguousarray(inputs["eps"], dtype=np.float32)

    def T(a):
        return np.ascontiguousarray(np.asarray(a, dtype=np.float32).T)

    import ml_dtypes
    wih = T(inputs["w_ih"])                     # [F, 4D]
    whh = T(inputs["w_hh"]).astype(ml_dtypes.bfloat16)    # [D, 4D]
    outw = T(inputs["out_w"]).astype(ml_dtypes.bfloat16)  # [D, F]
    zw = np.ascontiguousarray(0.5 * np.asarray(inputs["z_w"], np.float32).T)  # [F, 2F]
    # gelu on device is computed as 2*gelu (x*(1+erf)); fold the 0.5 into the
    # consumer weights: mlp_w2 and mlp_w3 each consume a 2*gelu output.
    mlp = np.concatenate([T(inputs["mlp_w1"]), 0.5 * T(inputs["mlp_w2"]),
                          0.5 * T(inputs["mlp_w3"]), T(inputs["hid_w"])], axis=1)

    gb = np.asarray(inputs["b_ih"], np.float32) + np.asarray(inputs["b_hh"], np.float32)
    out_b = np.asarray(inputs["out_b"], np.float32)
    z_b = np.asarray(inputs["z_b"], np.float32)
    mlp_b = [np.asarray(inputs[f"mlp_b{i}"], np.float32) for i in (1, 2, 3)]
    hid_b = np.asarray(inputs["hid_b"], np.float32)

    bias = np.zeros((F, 16), np.float32)
    bias[:, 0:8] = gb.reshape(8, F).T
    bias[:, 8] = out_b
    bias[:, 9] = z_b[:F]
    bias[:, 10] = 0.5 * z_b[F:]
    for i in range(3):
        bias[:, 11 + i] = mlp_b[i]
    bias[:, 14:16] = hid_b.reshape(2, F).T

    bias_flags = (
        bool(np.any(gb)), bool(np.any(out_b)), bool(np.any(z_b[:F])),
        bool(np.any(z_b[F:])),
        bool(any(np.any(b) for b in mlp_b)), bool(np.any(hid_b)),
    )

    S = eps.shape[0]
    in_maps = []
    for c in range(NCORES):
        sl = slice(c * BL, (c + 1) * BL)
        epsT = np.ascontiguousarray(eps[:, sl, :].transpose(0, 2, 1))  # [S,F,BL]
        noiseT = np.ascontiguousarray(noise[sl].T)                     # [F,BL]
        in_maps.append(dict(
            epsT=epsT, noiseT=noiseT, wih=wih, whh=whh, outw=outw,
            zw=zw, mlp=mlp, biaspack=bias,
        ))
    return in_maps, bias_flags, S


_CACHE = {}


def _get_nc(S, bias_flags):
    key = (S, bias_flags)
    if key not in _CACHE:
        _CACHE[key] = _build(S, bias_flags)
    return _CACHE[key]


def kernel(**inputs) -> np.ndarray:
    from concourse.bass_utils import run_bass_kernel_spmd

    in_maps, bias_flags, S = _prep_host(inputs)
    nc = _get_nc(S, bias_flags)
    res = run_bass_kernel_spmd(nc, in_maps, core_ids=list(range(NCORES)))
    outs = []
    for c in range(NCORES):
        o = res.results[c]["outT"]              # [S, F, BL]
        outs.append(np.ascontiguousarray(o.transpose(2, 0, 1)))  # [BL,S,F]
    return np.concatenate(outs, axis=0)        # [B, S, F]

